# revision 35
# baseline (speedup 1.0000x reference)
"""Gated multi-head attention on 8 NeuronCores.

Sharding (hardcoded): core c -> (batch b = c // 4, head-group g = c % 4).
Data-parallel over B=2, tensor-parallel over the 16 heads in groups of 4.
Each core computes its 4 heads' attention plus the corresponding slice of
the output projection; the host sums the 4 head-group partials per batch
(bf16 partials) and adds the output bias.

Per-core kernel (bf16 matmul inputs, fp32 PSUM):
  kT[256,2048], qT[256,2048] projections as before (gate sigmoid/sqrt(D)
  folded into the q scale, biases into the DVE PSUM->SBUF eviction).
  v[2048 pos, 4 heads, 65] with a ones column per head (denominator).

  Attention runs per (512-query block, head pair), heads alternating:
    QK^T: per kc chunk one matmul [K=64, M=128 kpos, F=512] into an
      S^T psum tile [128, 2, 512] (two kc chunks side by side); the two
      heads of a pair row-pack the PE array (rows 0-63 / 64-127).
    exp on ACT as [128,1024] activations (no max-subtraction).
    AV V-stationary: lhsT = v[kc][h] [128, 65], rhs = exp(S^T) [128,512]
      -> A^T psum [65 (64 dims + denom), 512 q] accumulated over 16 kc.
      F=512 streaming instead of the F=65 of the A-layout — 4x fewer
      PE rows and no PE transposes before the output projection.
    normalize: DVE reciprocal of the denom row -> DRAM bounce ->
      partition-broadcast DMA -> DVE multiply evicts A^T to SBUF bf16;
      odd heads land at partitions 64-127 via a small DMA shift.
  O-proj per 128-q chunk straight from the A^T tiles (no transposes);
  y stored bf16, summed on host in f32.
"""

import math
from collections import deque
from contextlib import ExitStack

import numpy as np

import concourse.bass as bass
import concourse.tile as tile
from concourse import mybir
from concourse.bass_utils import run_bass_kernel_spmd



B = 2
N = 2048
E = 1024
H = 16
D = 64
NCORES = 8
GROUPS = NCORES // B      # head-groups per batch
HG = H // GROUPS          # heads per core
DH = HG * D               # 256 head-dims per core
P = 128

F32 = mybir.dt.float32
BF16 = mybir.dt.bfloat16
AF = mybir.ActivationFunctionType
ALU = mybir.AluOpType

TRACE = False
LAST_RESULTS = None

KC = E // P            # 8 contraction chunks over the embed dim
MC = DH // P           # 2 partition chunks over this core's head dims
NB = N // 512          # 4 blocks of 512 positions
KB = N // P            # 16 key-position chunks
NPAIR = HG // 2        # head pairs per core
NQB = N // 512         # query 512-blocks


def _split_drain_waits(nc):
    """The installed walrus build accepts only ONE sync-wait per instruction
    (one NEURON_ISA_TPB_EVENTS slot), but Tile emits several on drains,
    matmuls, etc.  Hoist all but the last wait onto dedicated single-wait
    NOPs ahead of the instruction on the same engine (the lowering newer
    walrus performs itself)."""
    n = 0
    for fn in nc.m.functions:
        for bb in fn.blocks:
            insts = bb.instructions
            idx = 0
            while idx < len(insts):
                inst = insts[idx]
                si = inst.sync_info
                if si is not None and len(si.on_wait) > 1:
                    waits = list(si.on_wait)
                    nops = []
                    for w in waits[:-1]:
                        n += 1
                        nop = mybir.InstNoOp(
                            name=f"waitsplit-{n}",
                            engine=inst.engine,
                            sync_info=mybir.SyncInfo(on_wait=[w], on_update=[]),
                            bass_nofuse=True,
                        )
                        nc.register_instruction(nop)
                        nops.append(nop)
                    inst.sync_info = mybir.SyncInfo(
                        on_wait=[waits[-1]], on_update=list(si.on_update))
                    insts[idx:idx] = nops
                    idx += len(nops)
                idx += 1
    return n


def _build(reps=1):
    nc = bass.Bass()
    # inputs arrive partition-major ([128, chunk, free]) so each loads as a
    # single DMA with 32KB contiguous per-partition runs
    xqT = nc.dram_tensor("xqT", [P, KC, N], BF16, kind="ExternalInput")
    xkT = nc.dram_tensor("xkT", [P, KC, N], BF16, kind="ExternalInput")
    xvT = nc.dram_tensor("xvT", [P, KC, N], BF16, kind="ExternalInput")
    wqT = nc.dram_tensor("wqT", [P, KC, DH], BF16, kind="ExternalInput")
    wkT = nc.dram_tensor("wkT", [P, KC, DH], BF16, kind="ExternalInput")
    wvT = nc.dram_tensor("wvT", [P, KC, DH], BF16, kind="ExternalInput")
    woB = nc.dram_tensor("woB", [P, MC, E], BF16, kind="ExternalInput")
    qscale = nc.dram_tensor("qscale", [DH], F32, kind="ExternalInput")
    qbias = nc.dram_tensor("qbias", [DH], F32, kind="ExternalInput")
    kbias = nc.dram_tensor("kbias", [DH], F32, kind="ExternalInput")
    vbias = nc.dram_tensor("vbias", [DH], F32, kind="ExternalInput")
    y = nc.dram_tensor("y", [N, E], BF16, kind="ExternalOutput")
    # scratch rows for the denominator-reciprocal partition broadcast:
    # scr holds raw denominators, scr2 their reciprocals
    scr = nc.dram_tensor("scr", [NQB * HG, 512], F32, kind="Internal")
    scr2 = nc.dram_tensor("scr2", [NQB * HG, 512], F32, kind="Internal")

    with ExitStack() as ctx:
        tc = ctx.enter_context(tile.TileContext(nc))
        const = ctx.enter_context(tc.tile_pool(name="const", bufs=1))
        xpool = ctx.enter_context(tc.tile_pool(name="xpool", bufs=3))
        wpool = ctx.enter_context(tc.tile_pool(name="wpool", bufs=2))
        wqpool = ctx.enter_context(tc.tile_pool(name="wqpool", bufs=1))
        wopool = ctx.enter_context(tc.tile_pool(name="wopool", bufs=1))
        qkpool = ctx.enter_context(tc.tile_pool(name="qkpool", bufs=4))
        vpool = ctx.enter_context(tc.tile_pool(name="vpool", bufs=KB))
        ptpool = ctx.enter_context(tc.tile_pool(name="ptpool", bufs=12))
        attpool = ctx.enter_context(tc.tile_pool(name="attpool", bufs=4))
        tmppool = ctx.enter_context(tc.tile_pool(name="tmppool", bufs=2))
        stgpool = ctx.enter_context(tc.tile_pool(name="stgpool", bufs=4))
        rpool = ctx.enter_context(tc.tile_pool(name="rpool", bufs=2))
        dpool = ctx.enter_context(tc.tile_pool(name="dpool", bufs=4))
        bcpool = ctx.enter_context(tc.tile_pool(name="bcpool", bufs=2))
        ypool = ctx.enter_context(tc.tile_pool(name="ypool", bufs=2))
        # PSUM: stq 2x2 banks + pav 2x1 + ppp 2x1 -> exactly the 8 banks
        pstq = ctx.enter_context(tc.tile_pool(name="pstq", bufs=2,
                                              space="PSUM"))
        pav = ctx.enter_context(tc.tile_pool(name="pav", bufs=2,
                                             space="PSUM"))
        ppp = ctx.enter_context(tc.tile_pool(name="ppp", bufs=2,
                                             space="PSUM"))

        # ---- constant DMAs off the critical queue ----
        qs_sb = const.tile([P, MC], F32, name="qs")
        nc.scalar.dma_start(out=qs_sb,
                            in_=qscale[:].rearrange("(c p) -> p c", p=P))
        qb_sb = const.tile([P, MC], F32, name="qb")
        nc.scalar.dma_start(out=qb_sb,
                            in_=qbias[:].rearrange("(c p) -> p c", p=P))
        kb_sb = const.tile([P, MC], F32, name="kb")
        nc.scalar.dma_start(out=kb_sb,
                            in_=kbias[:].rearrange("(c p) -> p c", p=P))
        vb_ap = vbias[:]
        vb_bc = const.tile([P, DH], F32, name="vb")
        nc.gpsimd.dma_start(out=vb_bc, in_=bass.AP(
            tensor=vb_ap.tensor, offset=vb_ap.offset, ap=[[0, P]] + vb_ap.ap))


        def load_x(x_dram, engine):
            # 4 pieces into one tile: consumers start before the tail lands
            t = xpool.tile([P, KC, N], BF16, name="xs")
            for i in range(4):
                engine.dma_start(out=t[:, 2 * i:2 * i + 2, :],
                                 in_=x_dram[:, 2 * i:2 * i + 2, :])
            return [t[:, kc, :] for kc in range(KC)]

        def load_w(w_dram, pool, tag, engine):
            t = pool.tile([P, KC, DH], BF16, name=tag)
            engine.dma_start(out=t, in_=w_dram[:, :, :])
            return [t[:, kc, :] for kc in range(KC)]

        def emit_body():
            # x loads serialized on the SP queue in consumption order;
            # weights on the ACT / gpsimd queues
            xk = load_x(xkT, nc.sync)
            xq = load_x(xqT, nc.sync)
            xv = load_x(xvT, nc.sync)
            wk_c = load_w(wkT, wpool, "ws", nc.scalar)
            wq_c = load_w(wqT, wqpool, "wq", nc.scalar)
            wv_c = load_w(wvT, wpool, "ws", nc.gpsimd)
            wo_t = wopool.tile([P, MC, E], BF16, name="wo")
            nc.gpsimd.dma_start(out=wo_t, in_=woB[:, :, :])
            wo_sb = [wo_t[:, c, :] for c in range(MC)]

            # ---- projections (evictions on DVE) ----
            kT = [qkpool.tile([P, N], BF16, name="kt") for _ in range(MC)]
            qT = [qkpool.tile([P, N], BF16, name="qt") for _ in range(MC)]

            def proj_evict(pt, out_sb, c, nb, scale_sb, bias_sb):
                if scale_sb is None:
                    nc.vector.tensor_scalar_add(
                        out=out_sb[c][:, nb * 512:(nb + 1) * 512], in0=pt,
                        scalar1=bias_sb[:, c:c + 1])
                else:
                    nc.vector.tensor_scalar(
                        out=out_sb[c][:, nb * 512:(nb + 1) * 512], in0=pt,
                        scalar1=scale_sb[:, c:c + 1], scalar2=bias_sb[:, c:c + 1],
                        op0=ALU.mult, op1=ALU.add)

            def proj_group(xs, w_c, out_sb, c, nb, scale_sb, bias_sb):
                pt = ppp.tile([P, 512], F32, name="pp")
                for kc in range(KC):
                    nc.tensor.matmul(
                        pt,
                        lhsT=w_c[kc][:, c * P:(c + 1) * P],
                        rhs=xs[kc][:, nb * 512:(nb + 1) * 512],
                        start=(kc == 0), stop=(kc == KC - 1))
                proj_evict(pt, out_sb, c, nb, scale_sb, bias_sb)

            def proj_wave(xs, w_c, out_sb, groups, scale_sb, bias_sb, pools):
                # kc-outer so the matmuls chase the arriving x DMA pieces.
                # pstq-pool tiles keep their native [P, 2, 512] shape (mixed
                # sizes would fragment the pool); the wave uses half of each.
                pts = [pool.tile([P, 512], F32, name="pp") if pool is ppp
                       else pool.tile([P, 2, 512], F32, name="stq")[:, 0, :]
                       for pool in pools]
                for kc in range(KC):
                    for g, (c, nb) in enumerate(groups):
                        nc.tensor.matmul(
                            pts[g],
                            lhsT=w_c[kc][:, c * P:(c + 1) * P],
                            rhs=xs[kc][:, nb * 512:(nb + 1) * 512],
                            start=(kc == 0), stop=(kc == KC - 1))
                for g, (c, nb) in enumerate(groups):
                    proj_evict(pts[g], out_sb, c, nb, scale_sb, bias_sb)

            v_sb = []

            def vproj_group(m):
                vt = vpool.tile([P, HG, D + 1], BF16, name="vt")
                nc.gpsimd.memset(vt[:, :, D:D + 1], 1.0)
                pv = ppp.tile([P, 512], F32, name="pp")[:, :DH]
                for kc in range(KC):
                    nc.tensor.matmul(
                        pv,
                        lhsT=xv[kc][:, m * P:(m + 1) * P],
                        rhs=wv_c[kc],
                        start=(kc == 0), stop=(kc == KC - 1))
                nc.vector.tensor_add(
                    out=vt[:, :, 0:D],
                    in0=pv.rearrange("p (h d) -> p h d", h=HG),
                    in1=vb_bc.rearrange("p (h d) -> p h d", h=HG))
                v_sb.append(vt)

            # K proj fully, then the first Q proj block; the remaining Q
            # blocks are fills popped during attention
            for c in range(MC):
                for nb in range(NB):
                    proj_group(xk, wk_c, kT, c, nb, None, kb_sb)
            for c in range(MC):
                proj_group(xq, wq_c, qT, c, 0, qs_sb, qb_sb)

            kq_fills = deque()
            for nb in range(1, NB):
                for c in range(MC):
                    kq_fills.append(lambda c=c, nb=nb: proj_group(
                        xq, wq_c, qT, c, nb, qs_sb, qb_sb))
            o_fills = deque()

            # ---- attention: units of (512-query block, head pair) ----
            units = [(qb, pr) for qb in range(NQB) for pr in range(NPAIR)]
            att_tiles = {}          # qb -> [att_c2 tile for c2 in MC]

            def get_att(qb):
                if qb not in att_tiles:
                    att_tiles[qb] = [attpool.tile([P, 512], BF16, name="att")
                                     for _ in range(MC)]
                return att_tiles[qb]

            def emit_unit(u, first, last=False):
                qb, pr = u
                q0 = qb * 512
                avp = {}
                pend = deque()      # AV groups pipelined one kc-pair behind
                for h2 in range(2):     # head pr*2 + h2 at partitions 64*h2
                    avp[h2] = pav.tile([P, 512], F32, name="av")

                def emit_av(j, h2, ptile):
                    h = 2 * pr + h2
                    for t in range(2):
                        kc = 2 * j + t
                        nc.tensor.matmul(
                            avp[h2][0:D + 1, :],
                            lhsT=v_sb[kc][:, h, :],
                            rhs=ptile[:, t, :],
                            start=(kc == 0), stop=(kc == KB - 1),
                            skip_group_check=True)

                for j in range(KB // 2):    # kc pair (2j, 2j+1)
                    if first:
                        # V proj must stay ahead of the AV consumption
                        vproj_group(2 * j)
                        vproj_group(2 * j + 1)
                    for h2 in range(2):
                        off = h2 * D
                        stq = pstq.tile([P, 2, 512], F32, name="stq")
                        for t in range(2):
                            kc = 2 * j + t
                            nc.tensor.matmul(
                                stq[:, t, :],
                                lhsT=kT[pr][off:off + D, kc * P:(kc + 1) * P],
                                rhs=qT[pr][off:off + D, q0:q0 + 512],
                                start=True, stop=True)
                        ptile = ptpool.tile([P, 2, 512], BF16, name="pt")
                        nc.scalar.activation(out=ptile, in_=stq, func=AF.Exp)
                        pend.append((j, h2, ptile))
                    # AVs of the previous kc pair: their exps completed
                    # while this pair's QK matmuls streamed
                    while len(pend) > 2:
                        emit_av(*pend.popleft())
                    if not first:
                        pop_fills(1)
                while pend:
                    emit_av(*pend.popleft())
                if last:
                    # while the eviction chain's latency plays out, the PE
                    # runs the c2=0 halves of this qb's output projection
                    # (att[0] was evicted during this unit's attention),
                    # accumulating into the now-idle attention PSUM banks
                    oparts = emit_oproj_part1(qb)
                # normalize + evict A^T (denominator row D).  Stage both the
                # dims and the denom row out of PSUM immediately (releases
                # the accumulation bank); the reciprocal runs 128-lane via a
                # DRAM reshape bounce, then a partition-broadcast DMA feeds
                # the normalizing multiply.
                att = get_att(qb)[pr]
                sgs, rts = {}, {}
                for h2 in range(2):
                    sg = stgpool.tile([D, 512], BF16, name="sg")
                    nc.vector.tensor_copy(out=sg, in_=avp[h2][0:D, :])
                    rt = rpool.tile([1, 512], F32, name="rt")
                    nc.vector.tensor_copy(out=rt, in_=avp[h2][D:D + 1, :])
                    sgs[h2], rts[h2] = sg, rt
                for h2 in range(2):
                    row = qb * HG + pr * 2 + h2
                    nc.gpsimd.dma_start(out=scr[row, :], in_=rts[h2])
                    dn = dpool.tile([P, 4], F32, name="dn")
                    nc.gpsimd.dma_start(
                        out=dn, in_=scr[row, :].rearrange("(p j) -> p j", p=P))
                    rc = dpool.tile([P, 4], F32, name="rc")
                    nc.vector.reciprocal(out=rc, in_=dn)
                    nc.scalar.dma_start(
                        out=scr2[row, :].rearrange("(p j) -> p j", p=P), in_=rc)
                    bc = bcpool.tile([D, 512], F32, name="bc")
                    sap = scr2[row, :]
                    nc.scalar.dma_start(out=bc, in_=bass.AP(
                        tensor=sap.tensor, offset=sap.offset,
                        ap=[[0, D]] + sap.ap))
                    if h2 == 0:
                        nc.vector.tensor_tensor(
                            out=att[0:D, :], in0=sgs[h2], in1=bc, op=ALU.mult)
                    else:
                        tmp = tmppool.tile([D, 512], BF16, name="tmp")
                        nc.vector.tensor_tensor(
                            out=tmp, in0=sgs[h2], in1=bc, op=ALU.mult)
                        nc.sync.dma_start(out=att[D:P, :], in_=tmp)
                if last:
                    emit_oproj_part2(qb, oparts)

            def emit_oproj_part1(qb):
                # c2=0 partial accumulations for all 8 (qc, nn) blocks of
                # this qb, spread over the ppp/pav/pstq pools (the attention
                # banks are idle once the final AVs and exps retire)
                att0 = get_att(qb)[0]
                oparts = []
                holders = [ppp.tile([P, 512], F32, name="pp"),
                           ppp.tile([P, 512], F32, name="pp"),
                           pav.tile([P, 512], F32, name="av"),
                           pav.tile([P, 512], F32, name="av")]
                for t2 in range(2):
                    st = pstq.tile([P, 2, 512], F32, name="stq")
                    holders.append(st[:, 0, :])
                    holders.append(st[:, 1, :])
                for i, (qc, nn) in enumerate(
                        (qc, nn) for qc in range(4) for nn in range(2)):
                    pt = holders[i]
                    nc.tensor.matmul(
                        pt, lhsT=att0[:, qc * P:(qc + 1) * P],
                        rhs=wo_sb[0][:, nn * 512:(nn + 1) * 512],
                        start=True, stop=False, skip_group_check=True)
                    oparts.append((qc, nn, pt))
                return oparts

            def emit_oproj_part2(qb, oparts):
                att1 = get_att(qb)[1]
                for qc, nn, pt in oparts:
                    nc.tensor.matmul(
                        pt, lhsT=att1[:, qc * P:(qc + 1) * P],
                        rhs=wo_sb[1][:, nn * 512:(nn + 1) * 512],
                        start=False, stop=True, skip_group_check=True)
                # evict per (qc, nn): ACT is idle after the final exps
                ydone = {}
                for qc, nn, pt in oparts:
                    yt = ydone.setdefault(
                        qc, ypool.tile([P, E], BF16, name="yt"))
                    nc.scalar.copy(out=yt[:, nn * 512:(nn + 1) * 512], in_=pt)
                for qc, yt in ydone.items():
                    q0 = qb * 512 + qc * P
                    nc.sync.dma_start(out=y[q0:q0 + P, :], in_=yt)

            def emit_oproj(qb, qc, tail=False):
                att = get_att(qb)
                yt = ypool.tile([P, E], BF16, name="yt")
                for nn in range(2):
                    py = ppp.tile([P, 512], F32, name="pp")
                    for c2 in range(MC):
                        nc.tensor.matmul(
                            py, lhsT=att[c2][:, qc * P:(qc + 1) * P],
                            rhs=wo_sb[c2][:, nn * 512:(nn + 1) * 512],
                            start=(c2 == 0), stop=(c2 == MC - 1))
                    if tail:    # ACT is idle once the last exp has issued
                        nc.scalar.copy(out=yt[:, nn * 512:(nn + 1) * 512], in_=py)
                    else:
                        nc.vector.tensor_copy(
                            out=yt[:, nn * 512:(nn + 1) * 512], in_=py)
                q0 = qb * 512 + qc * P
                nc.sync.dma_start(out=y[q0:q0 + P, :], in_=yt)

            def pop_fills(budget):
                while budget > 0:
                    if kq_fills:
                        kq_fills.popleft()()
                    elif o_fills:
                        o_fills.popleft()()
                    else:
                        break
                    budget -= 1

            for i, u in enumerate(units):
                emit_unit(u, first=(i == 0), last=(i == len(units) - 1))
                qb, pr = u
                if pr == NPAIR - 1 and qb != NQB - 1:
                    for qc in range(4):
                        o_fills.append(
                            lambda qb=qb, qc=qc: emit_oproj(qb, qc))
            while kq_fills or o_fills:
                pop_fills(1)

        for _ in range(reps):
            emit_body()

    _split_drain_waits(nc)
    return nc


_CACHE = {}


def _get_nc():
    if "nc" not in _CACHE:
        _CACHE["nc"] = _build()
    return _CACHE["nc"]


def make_in_maps(query, key, value, Wq, bq, Wk, bk, Wv, bv, Wo, bo, gate):
    query = np.asarray(query, np.float32)
    key = np.asarray(key, np.float32)
    value = np.asarray(value, np.float32)
    Wq = np.asarray(Wq, np.float32)
    Wk = np.asarray(Wk, np.float32)
    Wv = np.asarray(Wv, np.float32)
    Wo = np.asarray(Wo, np.float32)
    bq = np.asarray(bq, np.float32)
    bk = np.asarray(bk, np.float32)
    bv = np.asarray(bv, np.float32)
    gate = np.asarray(gate, np.float32)

    scale_h = (1.0 / (1.0 + np.exp(-gate.astype(np.float64)))
               / math.sqrt(D)).astype(np.float32)

    def pmajor(a, np_, p=P):
        # [np_*p, free] -> [p, np_, free] (partition-major for 1-DMA loads)
        return np.ascontiguousarray(
            a.reshape(np_, p, a.shape[1]).transpose(1, 0, 2))

    xq_b = [pmajor(query[b].T, KC) for b in range(B)]
    xk_b = [pmajor(key[b].T, KC) for b in range(B)]
    xv_b = [pmajor(value[b].T, KC) for b in range(B)]

    in_maps = []
    for core in range(NCORES):
        b, g = divmod(core, GROUPS)
        rows = slice(g * DH, (g + 1) * DH)
        qs = np.repeat(scale_h[g * HG:(g + 1) * HG], D)
        in_maps.append({
            "xqT": xq_b[b], "xkT": xk_b[b], "xvT": xv_b[b],
            "wqT": pmajor(Wq[rows].T, KC),
            "wkT": pmajor(Wk[rows].T, KC),
            "wvT": pmajor(Wv[rows].T, KC),
            "woB": pmajor(Wo[:, rows].T, MC),
            "qscale": np.ascontiguousarray(qs),
            "qbias": np.ascontiguousarray(bq[rows] * qs),
            "kbias": np.ascontiguousarray(bk[rows]),
            "vbias": np.ascontiguousarray(bv[rows]),
        })

    from concourse import mybir as _mb
    bf = _mb.dt.np(_mb.dt.bfloat16)
    for m in in_maps:
        for k in ("xqT", "xkT", "xvT", "wqT", "wkT", "wvT", "woB"):
            m[k] = m[k].astype(bf)
    return in_maps


def assemble(results, query, key, value, Wq, bq, Wk, bk, Wv, bv, Wo, bo, gate):
    bo = np.asarray(bo, np.float32)
    out = np.empty((B, N, E), np.float32)
    for b in range(B):
        acc = results[b * GROUPS]["y"].astype(np.float32)
        for g in range(1, GROUPS):
            acc = acc + results[b * GROUPS + g]["y"].astype(np.float32)
        out[b] = acc + bo
    return out


def kernel(**inputs):
    global LAST_RESULTS
    in_maps = make_in_maps(**inputs)
    res = run_bass_kernel_spmd(_get_nc(), in_maps, list(range(NCORES)),
                               trace=TRACE)
    LAST_RESULTS = res
    return assemble(res.results, **inputs)


# revision 36
# speedup vs baseline: 1.0933x; 1.0933x over previous
"""Gated multi-head attention on 8 NeuronCores.

Sharding (hardcoded): core c -> (batch b = c // 4, head-group g = c % 4).
Data-parallel over B=2, tensor-parallel over the 16 heads in groups of 4.
Each core computes its 4 heads' attention plus the corresponding slice of
the output projection; the host sums the 4 head-group partials per batch
(bf16 partials) and adds the output bias.

Per-core kernel (bf16 matmul inputs, fp32 PSUM):
  kT[256,2048], qT[256,2048] projections as before (gate sigmoid/sqrt(D)
  folded into the q scale, biases into the DVE PSUM->SBUF eviction).
  v[2048 pos, 4 heads, 65] with a ones column per head (denominator).

  Attention runs per (512-query block, head pair), heads alternating:
    QK^T: per kc chunk one matmul [K=64, M=128 kpos, F=512] into an
      S^T psum tile [128, 2, 512] (two kc chunks side by side); the two
      heads of a pair row-pack the PE array (rows 0-63 / 64-127).
    exp on ACT as [128,1024] activations (no max-subtraction).
    AV V-stationary: lhsT = v[kc][h] [128, 65], rhs = exp(S^T) [128,512]
      -> A^T psum [65 (64 dims + denom), 512 q] accumulated over 16 kc.
      F=512 streaming instead of the F=65 of the A-layout — 4x fewer
      PE rows and no PE transposes before the output projection.
    normalize: DVE reciprocal of the denom row -> DRAM bounce ->
      partition-broadcast DMA -> DVE multiply evicts A^T to SBUF bf16;
      odd heads land at partitions 64-127 via a small DMA shift.
  O-proj per 128-q chunk straight from the A^T tiles (no transposes);
  y stored bf16, summed on host in f32.
"""

import math
from collections import deque
from contextlib import ExitStack

import numpy as np

import concourse.bass as bass
import concourse.tile as tile
from concourse import mybir
from concourse.bass_utils import run_bass_kernel_spmd



B = 2
N = 2048
E = 1024
H = 16
D = 64
NCORES = 8
GROUPS = NCORES // B      # head-groups per batch
HG = H // GROUPS          # heads per core
DH = HG * D               # 256 head-dims per core
P = 128

F32 = mybir.dt.float32
BF16 = mybir.dt.bfloat16
AF = mybir.ActivationFunctionType
ALU = mybir.AluOpType

TRACE = False
LAST_RESULTS = None

KC = E // P            # 8 contraction chunks over the embed dim
MC = DH // P           # 2 partition chunks over this core's head dims
NB = N // 512          # 4 blocks of 512 positions
KB = N // P            # 16 key-position chunks
NPAIR = HG // 2        # head pairs per core
NQB = N // 512         # query 512-blocks


def _split_drain_waits(nc):
    """The installed walrus build accepts only ONE sync-wait per instruction
    (one NEURON_ISA_TPB_EVENTS slot), but Tile emits several on drains,
    matmuls, etc.  Hoist all but the last wait onto dedicated single-wait
    NOPs ahead of the instruction on the same engine (the lowering newer
    walrus performs itself)."""
    n = 0
    for fn in nc.m.functions:
        for bb in fn.blocks:
            insts = bb.instructions
            idx = 0
            while idx < len(insts):
                inst = insts[idx]
                si = inst.sync_info
                if si is not None and len(si.on_wait) > 1:
                    waits = list(si.on_wait)
                    nops = []
                    for w in waits[:-1]:
                        n += 1
                        nop = mybir.InstNoOp(
                            name=f"waitsplit-{n}",
                            engine=inst.engine,
                            sync_info=mybir.SyncInfo(on_wait=[w], on_update=[]),
                            bass_nofuse=True,
                        )
                        nc.register_instruction(nop)
                        nops.append(nop)
                    inst.sync_info = mybir.SyncInfo(
                        on_wait=[waits[-1]], on_update=list(si.on_update))
                    insts[idx:idx] = nops
                    idx += len(nops)
                idx += 1
    return n


def _build(reps=1):
    nc = bass.Bass()
    # inputs arrive partition-major ([128, chunk, free]) so each loads as a
    # single DMA with 32KB contiguous per-partition runs
    xqT = nc.dram_tensor("xqT", [P, KC, N], BF16, kind="ExternalInput")
    xkT = nc.dram_tensor("xkT", [P, KC, N], BF16, kind="ExternalInput")
    xvT = nc.dram_tensor("xvT", [P, KC, N], BF16, kind="ExternalInput")
    wqT = nc.dram_tensor("wqT", [P, KC, DH], BF16, kind="ExternalInput")
    wkT = nc.dram_tensor("wkT", [P, KC, DH], BF16, kind="ExternalInput")
    wvT = nc.dram_tensor("wvT", [P, KC, DH], BF16, kind="ExternalInput")
    woB = nc.dram_tensor("woB", [P, MC, E], BF16, kind="ExternalInput")
    qscale = nc.dram_tensor("qscale", [DH], F32, kind="ExternalInput")
    qbias = nc.dram_tensor("qbias", [DH], F32, kind="ExternalInput")
    kbias = nc.dram_tensor("kbias", [DH], F32, kind="ExternalInput")
    vbias = nc.dram_tensor("vbias", [DH], F32, kind="ExternalInput")
    y = nc.dram_tensor("y", [N, E], BF16, kind="ExternalOutput")
    # scratch rows for the denominator-reciprocal partition broadcast:
    # scr holds raw denominators, scr2 their reciprocals
    scr = nc.dram_tensor("scr", [NQB * HG, 512], F32, kind="Internal")
    scr2 = nc.dram_tensor("scr2", [NQB * HG, 512], F32, kind="Internal")

    with ExitStack() as ctx:
        tc = ctx.enter_context(tile.TileContext(nc))
        const = ctx.enter_context(tc.tile_pool(name="const", bufs=1))
        xpool = ctx.enter_context(tc.tile_pool(name="xpool", bufs=3))
        wpool = ctx.enter_context(tc.tile_pool(name="wpool", bufs=2))
        wqpool = ctx.enter_context(tc.tile_pool(name="wqpool", bufs=1))
        wopool = ctx.enter_context(tc.tile_pool(name="wopool", bufs=1))
        qkpool = ctx.enter_context(tc.tile_pool(name="qkpool", bufs=4))
        vpool = ctx.enter_context(tc.tile_pool(name="vpool", bufs=KB))
        ptpool = ctx.enter_context(tc.tile_pool(name="ptpool", bufs=12))
        attpool = ctx.enter_context(tc.tile_pool(name="attpool", bufs=4))
        tmppool = ctx.enter_context(tc.tile_pool(name="tmppool", bufs=2))
        stgpool = ctx.enter_context(tc.tile_pool(name="stgpool", bufs=4))
        rpool = ctx.enter_context(tc.tile_pool(name="rpool", bufs=2))
        dpool = ctx.enter_context(tc.tile_pool(name="dpool", bufs=4))
        bcpool = ctx.enter_context(tc.tile_pool(name="bcpool", bufs=2))
        ypool = ctx.enter_context(tc.tile_pool(name="ypool", bufs=2))
        # PSUM: stq 2x2 banks + pav 2x1 + ppp 2x1 -> exactly the 8 banks
        pstq = ctx.enter_context(tc.tile_pool(name="pstq", bufs=2,
                                              space="PSUM"))
        pav = ctx.enter_context(tc.tile_pool(name="pav", bufs=2,
                                             space="PSUM"))
        ppp = ctx.enter_context(tc.tile_pool(name="ppp", bufs=2,
                                             space="PSUM"))

        # ---- constant DMAs off the critical queue ----
        qs_sb = const.tile([P, MC], F32, name="qs")
        nc.scalar.dma_start(out=qs_sb,
                            in_=qscale[:].rearrange("(c p) -> p c", p=P))
        qb_sb = const.tile([P, MC], F32, name="qb")
        nc.scalar.dma_start(out=qb_sb,
                            in_=qbias[:].rearrange("(c p) -> p c", p=P))
        kb_sb = const.tile([P, MC], F32, name="kb")
        nc.scalar.dma_start(out=kb_sb,
                            in_=kbias[:].rearrange("(c p) -> p c", p=P))
        vb_ap = vbias[:]
        vb_bc = const.tile([P, DH], F32, name="vb")
        nc.gpsimd.dma_start(out=vb_bc, in_=bass.AP(
            tensor=vb_ap.tensor, offset=vb_ap.offset, ap=[[0, P]] + vb_ap.ap))


        def load_x(x_dram, engine):
            # 4 pieces into one tile: consumers start before the tail lands
            t = xpool.tile([P, KC, N], BF16, name="xs")
            for i in range(4):
                engine.dma_start(out=t[:, 2 * i:2 * i + 2, :],
                                 in_=x_dram[:, 2 * i:2 * i + 2, :])
            return [t[:, kc, :] for kc in range(KC)]

        def load_w(w_dram, pool, tag, engine):
            t = pool.tile([P, KC, DH], BF16, name=tag)
            engine.dma_start(out=t, in_=w_dram[:, :, :])
            return [t[:, kc, :] for kc in range(KC)]

        def emit_body():
            # x loads serialized on the SP queue in consumption order;
            # weights on the ACT / gpsimd queues
            xk = load_x(xkT, nc.sync)
            xq = load_x(xqT, nc.sync)
            xv = load_x(xvT, nc.sync)
            wk_c = load_w(wkT, wpool, "ws", nc.scalar)
            wq_c = load_w(wqT, wqpool, "wq", nc.scalar)
            wv_c = load_w(wvT, wpool, "ws", nc.gpsimd)
            wo_t = wopool.tile([P, MC, E], BF16, name="wo")
            nc.gpsimd.dma_start(out=wo_t, in_=woB[:, :, :])
            wo_sb = [wo_t[:, c, :] for c in range(MC)]

            # ---- projections (evictions on DVE) ----
            kT = [qkpool.tile([P, N], BF16, name="kt") for _ in range(MC)]
            qT = [qkpool.tile([P, N], BF16, name="qt") for _ in range(MC)]

            def proj_evict(pt, out_sb, c, nb, scale_sb, bias_sb):
                if scale_sb is None:
                    nc.vector.tensor_scalar_add(
                        out=out_sb[c][:, nb * 512:(nb + 1) * 512], in0=pt,
                        scalar1=bias_sb[:, c:c + 1])
                else:
                    nc.vector.tensor_scalar(
                        out=out_sb[c][:, nb * 512:(nb + 1) * 512], in0=pt,
                        scalar1=scale_sb[:, c:c + 1], scalar2=bias_sb[:, c:c + 1],
                        op0=ALU.mult, op1=ALU.add)

            def proj_group(xs, w_c, out_sb, c, nb, scale_sb, bias_sb):
                pt = ppp.tile([P, 512], F32, name="pp")
                for kc in range(KC):
                    nc.tensor.matmul(
                        pt,
                        lhsT=w_c[kc][:, c * P:(c + 1) * P],
                        rhs=xs[kc][:, nb * 512:(nb + 1) * 512],
                        start=(kc == 0), stop=(kc == KC - 1))
                proj_evict(pt, out_sb, c, nb, scale_sb, bias_sb)

            def proj_wave(xs, w_c, out_sb, groups, scale_sb, bias_sb, pools):
                # kc-outer so the matmuls chase the arriving x DMA pieces.
                # pstq-pool tiles keep their native [P, 2, 512] shape (mixed
                # sizes would fragment the pool); the wave uses half of each.
                pts = [pool.tile([P, 512], F32, name="pp") if pool is ppp
                       else pool.tile([P, 2, 512], F32, name="stq")[:, 0, :]
                       for pool in pools]
                for kc in range(KC):
                    for g, (c, nb) in enumerate(groups):
                        nc.tensor.matmul(
                            pts[g],
                            lhsT=w_c[kc][:, c * P:(c + 1) * P],
                            rhs=xs[kc][:, nb * 512:(nb + 1) * 512],
                            start=(kc == 0), stop=(kc == KC - 1))
                for g, (c, nb) in enumerate(groups):
                    proj_evict(pts[g], out_sb, c, nb, scale_sb, bias_sb)

            v_sb = []

            def vproj_group(m):
                vt = vpool.tile([P, HG, D + 1], BF16, name="vt")
                nc.gpsimd.memset(vt[:, :, D:D + 1], 1.0)
                pv = ppp.tile([P, 512], F32, name="pp")[:, :DH]
                for kc in range(KC):
                    nc.tensor.matmul(
                        pv,
                        lhsT=xv[kc][:, m * P:(m + 1) * P],
                        rhs=wv_c[kc],
                        start=(kc == 0), stop=(kc == KC - 1))
                nc.vector.tensor_add(
                    out=vt[:, :, 0:D],
                    in0=pv.rearrange("p (h d) -> p h d", h=HG),
                    in1=vb_bc.rearrange("p (h d) -> p h d", h=HG))
                v_sb.append(vt)

            # K proj fully, then the first Q proj block; the remaining Q
            # blocks are fills popped during attention
            for c in range(MC):
                for nb in range(NB):
                    proj_group(xk, wk_c, kT, c, nb, None, kb_sb)
            for c in range(MC):
                proj_group(xq, wq_c, qT, c, 0, qs_sb, qb_sb)

            kq_fills = deque()
            for nb in range(1, NB):
                for c in range(MC):
                    kq_fills.append(lambda c=c, nb=nb: proj_group(
                        xq, wq_c, qT, c, nb, qs_sb, qb_sb))
            o_fills = deque()

            # ---- attention: units of (512-query block, head pair) ----
            units = [(qb, pr) for qb in range(NQB) for pr in range(NPAIR)]
            att_tiles = {}          # qb -> [att_c2 tile for c2 in MC]

            def get_att(qb):
                if qb not in att_tiles:
                    att_tiles[qb] = [attpool.tile([P, 512], BF16, name="att")
                                     for _ in range(MC)]
                return att_tiles[qb]

            def emit_unit(u, first, last=False):
                qb, pr = u
                q0 = qb * 512
                avp = {}
                pend = deque()      # AV groups pipelined one kc-pair behind
                for h2 in range(2):     # head pr*2 + h2 at partitions 64*h2
                    avp[h2] = pav.tile([P, 512], F32, name="av")

                def emit_av(j, h2, ptile):
                    h = 2 * pr + h2
                    for t in range(2):
                        kc = 2 * j + t
                        nc.tensor.matmul(
                            avp[h2][0:D + 1, :],
                            lhsT=v_sb[kc][:, h, :],
                            rhs=ptile[:, t, :],
                            start=(kc == 0), stop=(kc == KB - 1),
                            skip_group_check=True)

                for j in range(KB // 2):    # kc pair (2j, 2j+1)
                    if first:
                        # V proj must stay ahead of the AV consumption
                        vproj_group(2 * j)
                        vproj_group(2 * j + 1)
                    for h2 in range(2):
                        off = h2 * D
                        stq = pstq.tile([P, 2, 512], F32, name="stq")
                        for t in range(2):
                            kc = 2 * j + t
                            nc.tensor.matmul(
                                stq[:, t, :],
                                lhsT=kT[pr][off:off + D, kc * P:(kc + 1) * P],
                                rhs=qT[pr][off:off + D, q0:q0 + 512],
                                start=True, stop=True)
                        ptile = ptpool.tile([P, 2, 512], BF16, name="pt")
                        nc.scalar.activation(out=ptile, in_=stq, func=AF.Exp)
                        pend.append((j, h2, ptile))
                    # AVs of the previous kc pair: their exps completed
                    # while this pair's QK matmuls streamed
                    while len(pend) > 2:
                        emit_av(*pend.popleft())
                    if not first:
                        pop_fills(1)
                while pend:
                    emit_av(*pend.popleft())
                if last:
                    # while the eviction chain's latency plays out, the PE
                    # runs the c2=0 halves of this qb's output projection
                    # (att[0] was evicted during this unit's attention),
                    # accumulating into the now-idle attention PSUM banks
                    oparts = emit_oproj_part1(qb)
                # normalize + evict A^T (denominator row D).  Stage both the
                # dims and the denom row out of PSUM immediately (releases
                # the accumulation bank); the reciprocal runs 128-lane via a
                # DRAM reshape bounce, then a partition-broadcast DMA feeds
                # the normalizing multiply.
                att = get_att(qb)[pr]
                sgs, rts = {}, {}
                for h2 in range(2):
                    sg = stgpool.tile([D, 512], BF16, name="sg")
                    nc.vector.tensor_copy(out=sg, in_=avp[h2][0:D, :])
                    rt = rpool.tile([1, 512], F32, name="rt")
                    nc.vector.tensor_copy(out=rt, in_=avp[h2][D:D + 1, :])
                    sgs[h2], rts[h2] = sg, rt
                for h2 in range(2):
                    row = qb * HG + pr * 2 + h2
                    nc.gpsimd.dma_start(out=scr[row, :], in_=rts[h2])
                    dn = dpool.tile([P, 4], F32, name="dn")
                    nc.gpsimd.dma_start(
                        out=dn, in_=scr[row, :].rearrange("(p j) -> p j", p=P))
                    rc = dpool.tile([P, 4], F32, name="rc")
                    nc.vector.reciprocal(out=rc, in_=dn)
                    nc.gpsimd.dma_start(
                        out=scr2[row, :].rearrange("(p j) -> p j", p=P), in_=rc)
                    bc = bcpool.tile([D, 512], F32, name="bc")
                    sap = scr2[row, :]
                    nc.gpsimd.dma_start(out=bc, in_=bass.AP(
                        tensor=sap.tensor, offset=sap.offset,
                        ap=[[0, D]] + sap.ap))
                    if h2 == 0:
                        nc.vector.tensor_tensor(
                            out=att[0:D, :], in0=sgs[h2], in1=bc, op=ALU.mult)
                    else:
                        tmp = tmppool.tile([D, 512], BF16, name="tmp")
                        nc.vector.tensor_tensor(
                            out=tmp, in0=sgs[h2], in1=bc, op=ALU.mult)
                        nc.sync.dma_start(out=att[D:P, :], in_=tmp)
                if last:
                    emit_oproj_part2(qb, oparts)

            def emit_oproj_part1(qb):
                # c2=0 partial accumulations for all 8 (qc, nn) blocks of
                # this qb, spread over the ppp/pav/pstq pools (the attention
                # banks are idle once the final AVs and exps retire)
                att0 = get_att(qb)[0]
                oparts = []
                holders = [ppp.tile([P, 512], F32, name="pp"),
                           ppp.tile([P, 512], F32, name="pp"),
                           pav.tile([P, 512], F32, name="av"),
                           pav.tile([P, 512], F32, name="av")]
                for t2 in range(2):
                    st = pstq.tile([P, 2, 512], F32, name="stq")
                    holders.append(st[:, 0, :])
                    holders.append(st[:, 1, :])
                for i, (qc, nn) in enumerate(
                        (qc, nn) for qc in range(4) for nn in range(2)):
                    pt = holders[i]
                    nc.tensor.matmul(
                        pt, lhsT=att0[:, qc * P:(qc + 1) * P],
                        rhs=wo_sb[0][:, nn * 512:(nn + 1) * 512],
                        start=True, stop=False, skip_group_check=True)
                    oparts.append((qc, nn, pt))
                return oparts

            def emit_oproj_part2(qb, oparts):
                att1 = get_att(qb)[1]
                for qc, nn, pt in oparts:
                    nc.tensor.matmul(
                        pt, lhsT=att1[:, qc * P:(qc + 1) * P],
                        rhs=wo_sb[1][:, nn * 512:(nn + 1) * 512],
                        start=False, stop=True, skip_group_check=True)
                # evict per (qc, nn): ACT is idle after the final exps
                ydone = {}
                for qc, nn, pt in oparts:
                    yt = ydone.setdefault(
                        qc, ypool.tile([P, E], BF16, name="yt"))
                    nc.scalar.copy(out=yt[:, nn * 512:(nn + 1) * 512], in_=pt)
                for qc, yt in ydone.items():
                    q0 = qb * 512 + qc * P
                    nc.sync.dma_start(out=y[q0:q0 + P, :], in_=yt)

            def emit_oproj(qb, qc, tail=False):
                att = get_att(qb)
                yt = ypool.tile([P, E], BF16, name="yt")
                for nn in range(2):
                    py = ppp.tile([P, 512], F32, name="pp")
                    for c2 in range(MC):
                        nc.tensor.matmul(
                            py, lhsT=att[c2][:, qc * P:(qc + 1) * P],
                            rhs=wo_sb[c2][:, nn * 512:(nn + 1) * 512],
                            start=(c2 == 0), stop=(c2 == MC - 1))
                    if tail:    # ACT is idle once the last exp has issued
                        nc.scalar.copy(out=yt[:, nn * 512:(nn + 1) * 512], in_=py)
                    else:
                        nc.vector.tensor_copy(
                            out=yt[:, nn * 512:(nn + 1) * 512], in_=py)
                q0 = qb * 512 + qc * P
                nc.sync.dma_start(out=y[q0:q0 + P, :], in_=yt)

            def pop_fills(budget):
                while budget > 0:
                    if kq_fills:
                        kq_fills.popleft()()
                    elif o_fills:
                        o_fills.popleft()()
                    else:
                        break
                    budget -= 1

            for i, u in enumerate(units):
                emit_unit(u, first=(i == 0), last=(i == len(units) - 1))
                qb, pr = u
                if pr == NPAIR - 1 and qb != NQB - 1:
                    for qc in range(4):
                        o_fills.append(
                            lambda qb=qb, qc=qc: emit_oproj(qb, qc))
            while kq_fills or o_fills:
                pop_fills(1)

        for _ in range(reps):
            emit_body()

    _split_drain_waits(nc)
    return nc


_CACHE = {}


def _get_nc():
    if "nc" not in _CACHE:
        _CACHE["nc"] = _build()
    return _CACHE["nc"]


def make_in_maps(query, key, value, Wq, bq, Wk, bk, Wv, bv, Wo, bo, gate):
    query = np.asarray(query, np.float32)
    key = np.asarray(key, np.float32)
    value = np.asarray(value, np.float32)
    Wq = np.asarray(Wq, np.float32)
    Wk = np.asarray(Wk, np.float32)
    Wv = np.asarray(Wv, np.float32)
    Wo = np.asarray(Wo, np.float32)
    bq = np.asarray(bq, np.float32)
    bk = np.asarray(bk, np.float32)
    bv = np.asarray(bv, np.float32)
    gate = np.asarray(gate, np.float32)

    scale_h = (1.0 / (1.0 + np.exp(-gate.astype(np.float64)))
               / math.sqrt(D)).astype(np.float32)

    def pmajor(a, np_, p=P):
        # [np_*p, free] -> [p, np_, free] (partition-major for 1-DMA loads)
        return np.ascontiguousarray(
            a.reshape(np_, p, a.shape[1]).transpose(1, 0, 2))

    xq_b = [pmajor(query[b].T, KC) for b in range(B)]
    xk_b = [pmajor(key[b].T, KC) for b in range(B)]
    xv_b = [pmajor(value[b].T, KC) for b in range(B)]

    in_maps = []
    for core in range(NCORES):
        b, g = divmod(core, GROUPS)
        rows = slice(g * DH, (g + 1) * DH)
        qs = np.repeat(scale_h[g * HG:(g + 1) * HG], D)
        in_maps.append({
            "xqT": xq_b[b], "xkT": xk_b[b], "xvT": xv_b[b],
            "wqT": pmajor(Wq[rows].T, KC),
            "wkT": pmajor(Wk[rows].T, KC),
            "wvT": pmajor(Wv[rows].T, KC),
            "woB": pmajor(Wo[:, rows].T, MC),
            "qscale": np.ascontiguousarray(qs),
            "qbias": np.ascontiguousarray(bq[rows] * qs),
            "kbias": np.ascontiguousarray(bk[rows]),
            "vbias": np.ascontiguousarray(bv[rows]),
        })

    from concourse import mybir as _mb
    bf = _mb.dt.np(_mb.dt.bfloat16)
    for m in in_maps:
        for k in ("xqT", "xkT", "xvT", "wqT", "wkT", "wvT", "woB"):
            m[k] = m[k].astype(bf)
    return in_maps


def assemble(results, query, key, value, Wq, bq, Wk, bk, Wv, bv, Wo, bo, gate):
    bo = np.asarray(bo, np.float32)
    out = np.empty((B, N, E), np.float32)
    for b in range(B):
        acc = results[b * GROUPS]["y"].astype(np.float32)
        for g in range(1, GROUPS):
            acc = acc + results[b * GROUPS + g]["y"].astype(np.float32)
        out[b] = acc + bo
    return out


def kernel(**inputs):
    global LAST_RESULTS
    in_maps = make_in_maps(**inputs)
    res = run_bass_kernel_spmd(_get_nc(), in_maps, list(range(NCORES)),
                               trace=TRACE)
    LAST_RESULTS = res
    return assemble(res.results, **inputs)


# revision 37
# speedup vs baseline: 1.1065x; 1.0121x over previous
"""Gated multi-head attention on 8 NeuronCores.

Sharding (hardcoded): core c -> (batch b = c // 4, head-group g = c % 4).
Data-parallel over B=2, tensor-parallel over the 16 heads in groups of 4.
Each core computes its 4 heads' attention plus the corresponding slice of
the output projection; the host sums the 4 head-group partials per batch
(bf16 partials) and adds the output bias.

Per-core kernel (bf16 matmul inputs, fp32 PSUM):
  kT[256,2048], qT[256,2048] projections as before (gate sigmoid/sqrt(D)
  folded into the q scale, biases into the DVE PSUM->SBUF eviction).
  v[2048 pos, 4 heads, 65] with a ones column per head (denominator).

  Attention runs per (512-query block, head pair), heads alternating:
    QK^T: per kc chunk one matmul [K=64, M=128 kpos, F=512] into an
      S^T psum tile [128, 2, 512] (two kc chunks side by side); the two
      heads of a pair row-pack the PE array (rows 0-63 / 64-127).
    exp on ACT as [128,1024] activations (no max-subtraction).
    AV V-stationary: lhsT = v[kc][h] [128, 65], rhs = exp(S^T) [128,512]
      -> A^T psum [65 (64 dims + denom), 512 q] accumulated over 16 kc.
      F=512 streaming instead of the F=65 of the A-layout — 4x fewer
      PE rows and no PE transposes before the output projection.
    normalize: DVE reciprocal of the denom row -> DRAM bounce ->
      partition-broadcast DMA -> DVE multiply evicts A^T to SBUF bf16;
      odd heads land at partitions 64-127 via a small DMA shift.
  O-proj per 128-q chunk straight from the A^T tiles (no transposes);
  y stored bf16, summed on host in f32.
"""

import math
from collections import deque
from contextlib import ExitStack

import numpy as np

import concourse.bass as bass
import concourse.tile as tile
from concourse import mybir
from concourse.bass_utils import run_bass_kernel_spmd



B = 2
N = 2048
E = 1024
H = 16
D = 64
NCORES = 8
GROUPS = NCORES // B      # head-groups per batch
HG = H // GROUPS          # heads per core
DH = HG * D               # 256 head-dims per core
P = 128

F32 = mybir.dt.float32
BF16 = mybir.dt.bfloat16
AF = mybir.ActivationFunctionType
ALU = mybir.AluOpType

TRACE = False
LAST_RESULTS = None

KC = E // P            # 8 contraction chunks over the embed dim
MC = DH // P           # 2 partition chunks over this core's head dims
NB = N // 512          # 4 blocks of 512 positions
KB = N // P            # 16 key-position chunks
NPAIR = HG // 2        # head pairs per core
NQB = N // 512         # query 512-blocks


def _split_drain_waits(nc):
    """The installed walrus build accepts only ONE sync-wait per instruction
    (one NEURON_ISA_TPB_EVENTS slot), but Tile emits several on drains,
    matmuls, etc.  Hoist all but the last wait onto dedicated single-wait
    NOPs ahead of the instruction on the same engine (the lowering newer
    walrus performs itself)."""
    n = 0
    for fn in nc.m.functions:
        for bb in fn.blocks:
            insts = bb.instructions
            idx = 0
            while idx < len(insts):
                inst = insts[idx]
                si = inst.sync_info
                if si is not None and len(si.on_wait) > 1:
                    waits = list(si.on_wait)
                    nops = []
                    for w in waits[:-1]:
                        n += 1
                        nop = mybir.InstNoOp(
                            name=f"waitsplit-{n}",
                            engine=inst.engine,
                            sync_info=mybir.SyncInfo(on_wait=[w], on_update=[]),
                            bass_nofuse=True,
                        )
                        nc.register_instruction(nop)
                        nops.append(nop)
                    inst.sync_info = mybir.SyncInfo(
                        on_wait=[waits[-1]], on_update=list(si.on_update))
                    insts[idx:idx] = nops
                    idx += len(nops)
                idx += 1
    return n


def _build(reps=1):
    nc = bass.Bass()
    # inputs arrive partition-major ([128, chunk, free]) so each loads as a
    # single DMA with 32KB contiguous per-partition runs
    xqT = nc.dram_tensor("xqT", [P, KC, N], BF16, kind="ExternalInput")
    xkT = nc.dram_tensor("xkT", [P, KC, N], BF16, kind="ExternalInput")
    xvT = nc.dram_tensor("xvT", [P, KC, N], BF16, kind="ExternalInput")
    wqT = nc.dram_tensor("wqT", [P, KC, DH], BF16, kind="ExternalInput")
    wkT = nc.dram_tensor("wkT", [P, KC, DH], BF16, kind="ExternalInput")
    wvT = nc.dram_tensor("wvT", [P, KC, DH], BF16, kind="ExternalInput")
    woB = nc.dram_tensor("woB", [P, MC, E], BF16, kind="ExternalInput")
    qscale = nc.dram_tensor("qscale", [DH], F32, kind="ExternalInput")
    qbias = nc.dram_tensor("qbias", [DH], F32, kind="ExternalInput")
    kbias = nc.dram_tensor("kbias", [DH], F32, kind="ExternalInput")
    vbias = nc.dram_tensor("vbias", [DH], F32, kind="ExternalInput")
    y = nc.dram_tensor("y", [N, E], BF16, kind="ExternalOutput")
    # scratch rows for the denominator-reciprocal partition broadcast:
    # scr holds raw denominators, scr2 their reciprocals
    scr = nc.dram_tensor("scr", [NQB * HG, 512], F32, kind="Internal")
    scr2 = nc.dram_tensor("scr2", [NQB * HG, 512], F32, kind="Internal")

    with ExitStack() as ctx:
        tc = ctx.enter_context(tile.TileContext(nc))
        const = ctx.enter_context(tc.tile_pool(name="const", bufs=1))
        xpool = ctx.enter_context(tc.tile_pool(name="xpool", bufs=3))
        wpool = ctx.enter_context(tc.tile_pool(name="wpool", bufs=2))
        wqpool = ctx.enter_context(tc.tile_pool(name="wqpool", bufs=1))
        wopool = ctx.enter_context(tc.tile_pool(name="wopool", bufs=1))
        qkpool = ctx.enter_context(tc.tile_pool(name="qkpool", bufs=4))
        vpool = ctx.enter_context(tc.tile_pool(name="vpool", bufs=KB))
        ptpool = ctx.enter_context(tc.tile_pool(name="ptpool", bufs=12))
        attpool = ctx.enter_context(tc.tile_pool(name="attpool", bufs=4))
        tmppool = ctx.enter_context(tc.tile_pool(name="tmppool", bufs=2))
        stgpool = ctx.enter_context(tc.tile_pool(name="stgpool", bufs=4))
        rpool = ctx.enter_context(tc.tile_pool(name="rpool", bufs=2))
        dpool = ctx.enter_context(tc.tile_pool(name="dpool", bufs=4))
        bcpool = ctx.enter_context(tc.tile_pool(name="bcpool", bufs=2))
        ypool = ctx.enter_context(tc.tile_pool(name="ypool", bufs=2))
        # PSUM: stq 2x2 banks + pav 2x1 + ppp 2x1 -> exactly the 8 banks
        pstq = ctx.enter_context(tc.tile_pool(name="pstq", bufs=2,
                                              space="PSUM"))
        pav = ctx.enter_context(tc.tile_pool(name="pav", bufs=2,
                                             space="PSUM"))
        ppp = ctx.enter_context(tc.tile_pool(name="ppp", bufs=2,
                                             space="PSUM"))

        # ---- constant DMAs off the critical queue ----
        qs_sb = const.tile([P, MC], F32, name="qs")
        nc.scalar.dma_start(out=qs_sb,
                            in_=qscale[:].rearrange("(c p) -> p c", p=P))
        qb_sb = const.tile([P, MC], F32, name="qb")
        nc.scalar.dma_start(out=qb_sb,
                            in_=qbias[:].rearrange("(c p) -> p c", p=P))
        kb_sb = const.tile([P, MC], F32, name="kb")
        nc.scalar.dma_start(out=kb_sb,
                            in_=kbias[:].rearrange("(c p) -> p c", p=P))
        vb_ap = vbias[:]
        vb_bc = const.tile([P, DH], F32, name="vb")
        nc.gpsimd.dma_start(out=vb_bc, in_=bass.AP(
            tensor=vb_ap.tensor, offset=vb_ap.offset, ap=[[0, P]] + vb_ap.ap))


        def load_x(x_dram, engine):
            # 4 pieces into one tile: consumers start before the tail lands
            t = xpool.tile([P, KC, N], BF16, name="xs")
            for i in range(4):
                engine.dma_start(out=t[:, 2 * i:2 * i + 2, :],
                                 in_=x_dram[:, 2 * i:2 * i + 2, :])
            return [t[:, kc, :] for kc in range(KC)]

        def load_w(w_dram, pool, tag, engine):
            t = pool.tile([P, KC, DH], BF16, name=tag)
            engine.dma_start(out=t, in_=w_dram[:, :, :])
            return [t[:, kc, :] for kc in range(KC)]

        def emit_body():
            # xk then xq serialized on the SP queue (K proj gates everything);
            # xv concurrent on the gpsimd queue; weights on the ACT queue
            xk = load_x(xkT, nc.sync)
            xq = load_x(xqT, nc.sync)
            xv = load_x(xvT, nc.gpsimd)
            wk_c = load_w(wkT, wpool, "ws", nc.scalar)
            wq_c = load_w(wqT, wqpool, "wq", nc.scalar)
            wv_c = load_w(wvT, wpool, "ws", nc.scalar)
            wo_t = wopool.tile([P, MC, E], BF16, name="wo")
            nc.scalar.dma_start(out=wo_t, in_=woB[:, :, :])
            wo_sb = [wo_t[:, c, :] for c in range(MC)]

            # ---- projections (evictions on DVE) ----
            kT = [qkpool.tile([P, N], BF16, name="kt") for _ in range(MC)]
            qT = [qkpool.tile([P, N], BF16, name="qt") for _ in range(MC)]

            def proj_evict(pt, out_sb, c, nb, scale_sb, bias_sb):
                if scale_sb is None:
                    nc.vector.tensor_scalar_add(
                        out=out_sb[c][:, nb * 512:(nb + 1) * 512], in0=pt,
                        scalar1=bias_sb[:, c:c + 1])
                else:
                    nc.vector.tensor_scalar(
                        out=out_sb[c][:, nb * 512:(nb + 1) * 512], in0=pt,
                        scalar1=scale_sb[:, c:c + 1], scalar2=bias_sb[:, c:c + 1],
                        op0=ALU.mult, op1=ALU.add)

            def proj_group(xs, w_c, out_sb, c, nb, scale_sb, bias_sb):
                pt = ppp.tile([P, 512], F32, name="pp")
                for kc in range(KC):
                    nc.tensor.matmul(
                        pt,
                        lhsT=w_c[kc][:, c * P:(c + 1) * P],
                        rhs=xs[kc][:, nb * 512:(nb + 1) * 512],
                        start=(kc == 0), stop=(kc == KC - 1))
                proj_evict(pt, out_sb, c, nb, scale_sb, bias_sb)

            def proj_wave(xs, w_c, out_sb, groups, scale_sb, bias_sb, pools):
                # kc-outer so the matmuls chase the arriving x DMA pieces.
                # pstq-pool tiles keep their native [P, 2, 512] shape (mixed
                # sizes would fragment the pool); the wave uses half of each.
                pts = [pool.tile([P, 512], F32, name="pp") if pool is ppp
                       else pool.tile([P, 2, 512], F32, name="stq")[:, 0, :]
                       for pool in pools]
                for kc in range(KC):
                    for g, (c, nb) in enumerate(groups):
                        nc.tensor.matmul(
                            pts[g],
                            lhsT=w_c[kc][:, c * P:(c + 1) * P],
                            rhs=xs[kc][:, nb * 512:(nb + 1) * 512],
                            start=(kc == 0), stop=(kc == KC - 1))
                for g, (c, nb) in enumerate(groups):
                    proj_evict(pts[g], out_sb, c, nb, scale_sb, bias_sb)

            v_sb = []

            def vproj_group(m):
                vt = vpool.tile([P, HG, D + 1], BF16, name="vt")
                nc.gpsimd.memset(vt[:, :, D:D + 1], 1.0)
                pv = ppp.tile([P, 512], F32, name="pp")[:, :DH]
                for kc in range(KC):
                    nc.tensor.matmul(
                        pv,
                        lhsT=xv[kc][:, m * P:(m + 1) * P],
                        rhs=wv_c[kc],
                        start=(kc == 0), stop=(kc == KC - 1))
                nc.vector.tensor_add(
                    out=vt[:, :, 0:D],
                    in0=pv.rearrange("p (h d) -> p h d", h=HG),
                    in1=vb_bc.rearrange("p (h d) -> p h d", h=HG))
                v_sb.append(vt)

            # K proj fully, then the first Q proj block; the remaining Q
            # blocks are fills popped during attention
            for c in range(MC):
                for nb in range(NB):
                    proj_group(xk, wk_c, kT, c, nb, None, kb_sb)
            for c in range(MC):
                proj_group(xq, wq_c, qT, c, 0, qs_sb, qb_sb)

            kq_fills = deque()
            for nb in range(1, NB):
                for c in range(MC):
                    kq_fills.append(lambda c=c, nb=nb: proj_group(
                        xq, wq_c, qT, c, nb, qs_sb, qb_sb))
            o_fills = deque()

            # ---- attention: units of (512-query block, head pair) ----
            units = [(qb, pr) for qb in range(NQB) for pr in range(NPAIR)]
            att_tiles = {}          # qb -> [att_c2 tile for c2 in MC]

            def get_att(qb):
                if qb not in att_tiles:
                    att_tiles[qb] = [attpool.tile([P, 512], BF16, name="att")
                                     for _ in range(MC)]
                return att_tiles[qb]

            def emit_unit(u, first, last=False):
                qb, pr = u
                q0 = qb * 512
                avp = {}
                pend = deque()      # AV groups pipelined one kc-pair behind
                for h2 in range(2):     # head pr*2 + h2 at partitions 64*h2
                    avp[h2] = pav.tile([P, 512], F32, name="av")

                def emit_av(j, h2, ptile):
                    h = 2 * pr + h2
                    for t in range(2):
                        kc = 2 * j + t
                        nc.tensor.matmul(
                            avp[h2][0:D + 1, :],
                            lhsT=v_sb[kc][:, h, :],
                            rhs=ptile[:, t, :],
                            start=(kc == 0), stop=(kc == KB - 1),
                            skip_group_check=True)

                for j in range(KB // 2):    # kc pair (2j, 2j+1)
                    if first:
                        # V proj must stay ahead of the AV consumption
                        vproj_group(2 * j)
                        vproj_group(2 * j + 1)
                    for h2 in range(2):
                        off = h2 * D
                        stq = pstq.tile([P, 2, 512], F32, name="stq")
                        for t in range(2):
                            kc = 2 * j + t
                            nc.tensor.matmul(
                                stq[:, t, :],
                                lhsT=kT[pr][off:off + D, kc * P:(kc + 1) * P],
                                rhs=qT[pr][off:off + D, q0:q0 + 512],
                                start=True, stop=True)
                        ptile = ptpool.tile([P, 2, 512], BF16, name="pt")
                        nc.scalar.activation(out=ptile, in_=stq, func=AF.Exp)
                        pend.append((j, h2, ptile))
                    # AVs of the previous kc pair: their exps completed
                    # while this pair's QK matmuls streamed
                    while len(pend) > 2:
                        emit_av(*pend.popleft())
                    if not first:
                        pop_fills(1)
                while pend:
                    emit_av(*pend.popleft())
                if last:
                    # while the eviction chain's latency plays out, the PE
                    # runs the c2=0 halves of this qb's output projection
                    # (att[0] was evicted during this unit's attention),
                    # accumulating into the now-idle attention PSUM banks
                    oparts = emit_oproj_part1(qb)
                # normalize + evict A^T (denominator row D).  Stage both the
                # dims and the denom row out of PSUM immediately (releases
                # the accumulation bank); the reciprocal runs 128-lane via a
                # DRAM reshape bounce, then a partition-broadcast DMA feeds
                # the normalizing multiply.
                att = get_att(qb)[pr]
                sgs, rts = {}, {}
                for h2 in range(2):
                    sg = stgpool.tile([D, 512], BF16, name="sg")
                    nc.vector.tensor_copy(out=sg, in_=avp[h2][0:D, :])
                    rt = rpool.tile([1, 512], F32, name="rt")
                    nc.vector.tensor_copy(out=rt, in_=avp[h2][D:D + 1, :])
                    sgs[h2], rts[h2] = sg, rt
                for h2 in range(2):
                    row = qb * HG + pr * 2 + h2
                    nc.gpsimd.dma_start(out=scr[row, :], in_=rts[h2])
                    dn = dpool.tile([P, 4], F32, name="dn")
                    nc.gpsimd.dma_start(
                        out=dn, in_=scr[row, :].rearrange("(p j) -> p j", p=P))
                    rc = dpool.tile([P, 4], F32, name="rc")
                    nc.vector.reciprocal(out=rc, in_=dn)
                    nc.gpsimd.dma_start(
                        out=scr2[row, :].rearrange("(p j) -> p j", p=P), in_=rc)
                    bc = bcpool.tile([D, 512], F32, name="bc")
                    sap = scr2[row, :]
                    nc.gpsimd.dma_start(out=bc, in_=bass.AP(
                        tensor=sap.tensor, offset=sap.offset,
                        ap=[[0, D]] + sap.ap))
                    if h2 == 0:
                        nc.vector.tensor_tensor(
                            out=att[0:D, :], in0=sgs[h2], in1=bc, op=ALU.mult)
                    else:
                        tmp = tmppool.tile([D, 512], BF16, name="tmp")
                        nc.vector.tensor_tensor(
                            out=tmp, in0=sgs[h2], in1=bc, op=ALU.mult)
                        nc.sync.dma_start(out=att[D:P, :], in_=tmp)
                if last:
                    emit_oproj_part2(qb, oparts)

            def emit_oproj_part1(qb):
                # c2=0 partial accumulations for all 8 (qc, nn) blocks of
                # this qb, spread over the ppp/pav/pstq pools (the attention
                # banks are idle once the final AVs and exps retire)
                att0 = get_att(qb)[0]
                oparts = []
                holders = [ppp.tile([P, 512], F32, name="pp"),
                           ppp.tile([P, 512], F32, name="pp"),
                           pav.tile([P, 512], F32, name="av"),
                           pav.tile([P, 512], F32, name="av")]
                for t2 in range(2):
                    st = pstq.tile([P, 2, 512], F32, name="stq")
                    holders.append(st[:, 0, :])
                    holders.append(st[:, 1, :])
                for i, (qc, nn) in enumerate(
                        (qc, nn) for qc in range(4) for nn in range(2)):
                    pt = holders[i]
                    nc.tensor.matmul(
                        pt, lhsT=att0[:, qc * P:(qc + 1) * P],
                        rhs=wo_sb[0][:, nn * 512:(nn + 1) * 512],
                        start=True, stop=False, skip_group_check=True)
                    oparts.append((qc, nn, pt))
                return oparts

            def emit_oproj_part2(qb, oparts):
                att1 = get_att(qb)[1]
                for qc, nn, pt in oparts:
                    nc.tensor.matmul(
                        pt, lhsT=att1[:, qc * P:(qc + 1) * P],
                        rhs=wo_sb[1][:, nn * 512:(nn + 1) * 512],
                        start=False, stop=True, skip_group_check=True)
                # evict per (qc, nn): ACT is idle after the final exps
                ydone = {}
                for qc, nn, pt in oparts:
                    yt = ydone.setdefault(
                        qc, ypool.tile([P, E], BF16, name="yt"))
                    nc.scalar.copy(out=yt[:, nn * 512:(nn + 1) * 512], in_=pt)
                for qc, yt in ydone.items():
                    q0 = qb * 512 + qc * P
                    nc.sync.dma_start(out=y[q0:q0 + P, :], in_=yt)

            def emit_oproj(qb, qc, tail=False):
                att = get_att(qb)
                yt = ypool.tile([P, E], BF16, name="yt")
                for nn in range(2):
                    py = ppp.tile([P, 512], F32, name="pp")
                    for c2 in range(MC):
                        nc.tensor.matmul(
                            py, lhsT=att[c2][:, qc * P:(qc + 1) * P],
                            rhs=wo_sb[c2][:, nn * 512:(nn + 1) * 512],
                            start=(c2 == 0), stop=(c2 == MC - 1))
                    if tail:    # ACT is idle once the last exp has issued
                        nc.scalar.copy(out=yt[:, nn * 512:(nn + 1) * 512], in_=py)
                    else:
                        nc.vector.tensor_copy(
                            out=yt[:, nn * 512:(nn + 1) * 512], in_=py)
                q0 = qb * 512 + qc * P
                nc.sync.dma_start(out=y[q0:q0 + P, :], in_=yt)

            def pop_fills(budget):
                while budget > 0:
                    if kq_fills:
                        kq_fills.popleft()()
                    elif o_fills:
                        o_fills.popleft()()
                    else:
                        break
                    budget -= 1

            for i, u in enumerate(units):
                emit_unit(u, first=(i == 0), last=(i == len(units) - 1))
                qb, pr = u
                if pr == NPAIR - 1 and qb != NQB - 1:
                    for qc in range(4):
                        o_fills.append(
                            lambda qb=qb, qc=qc: emit_oproj(qb, qc))
            while kq_fills or o_fills:
                pop_fills(1)

        for _ in range(reps):
            emit_body()

    _split_drain_waits(nc)
    return nc


_CACHE = {}


def _get_nc():
    if "nc" not in _CACHE:
        _CACHE["nc"] = _build()
    return _CACHE["nc"]


def make_in_maps(query, key, value, Wq, bq, Wk, bk, Wv, bv, Wo, bo, gate):
    query = np.asarray(query, np.float32)
    key = np.asarray(key, np.float32)
    value = np.asarray(value, np.float32)
    Wq = np.asarray(Wq, np.float32)
    Wk = np.asarray(Wk, np.float32)
    Wv = np.asarray(Wv, np.float32)
    Wo = np.asarray(Wo, np.float32)
    bq = np.asarray(bq, np.float32)
    bk = np.asarray(bk, np.float32)
    bv = np.asarray(bv, np.float32)
    gate = np.asarray(gate, np.float32)

    scale_h = (1.0 / (1.0 + np.exp(-gate.astype(np.float64)))
               / math.sqrt(D)).astype(np.float32)

    def pmajor(a, np_, p=P):
        # [np_*p, free] -> [p, np_, free] (partition-major for 1-DMA loads)
        return np.ascontiguousarray(
            a.reshape(np_, p, a.shape[1]).transpose(1, 0, 2))

    xq_b = [pmajor(query[b].T, KC) for b in range(B)]
    xk_b = [pmajor(key[b].T, KC) for b in range(B)]
    xv_b = [pmajor(value[b].T, KC) for b in range(B)]

    in_maps = []
    for core in range(NCORES):
        b, g = divmod(core, GROUPS)
        rows = slice(g * DH, (g + 1) * DH)
        qs = np.repeat(scale_h[g * HG:(g + 1) * HG], D)
        in_maps.append({
            "xqT": xq_b[b], "xkT": xk_b[b], "xvT": xv_b[b],
            "wqT": pmajor(Wq[rows].T, KC),
            "wkT": pmajor(Wk[rows].T, KC),
            "wvT": pmajor(Wv[rows].T, KC),
            "woB": pmajor(Wo[:, rows].T, MC),
            "qscale": np.ascontiguousarray(qs),
            "qbias": np.ascontiguousarray(bq[rows] * qs),
            "kbias": np.ascontiguousarray(bk[rows]),
            "vbias": np.ascontiguousarray(bv[rows]),
        })

    from concourse import mybir as _mb
    bf = _mb.dt.np(_mb.dt.bfloat16)
    for m in in_maps:
        for k in ("xqT", "xkT", "xvT", "wqT", "wkT", "wvT", "woB"):
            m[k] = m[k].astype(bf)
    return in_maps


def assemble(results, query, key, value, Wq, bq, Wk, bk, Wv, bv, Wo, bo, gate):
    bo = np.asarray(bo, np.float32)
    out = np.empty((B, N, E), np.float32)
    for b in range(B):
        acc = results[b * GROUPS]["y"].astype(np.float32)
        for g in range(1, GROUPS):
            acc = acc + results[b * GROUPS + g]["y"].astype(np.float32)
        out[b] = acc + bo
    return out


def kernel(**inputs):
    global LAST_RESULTS
    in_maps = make_in_maps(**inputs)
    res = run_bass_kernel_spmd(_get_nc(), in_maps, list(range(NCORES)),
                               trace=TRACE)
    LAST_RESULTS = res
    return assemble(res.results, **inputs)


# revision 38
# speedup vs baseline: 1.1092x; 1.0024x over previous
"""Gated multi-head attention on 8 NeuronCores.

Sharding (hardcoded): core c -> (batch b = c // 4, head-group g = c % 4).
Data-parallel over B=2, tensor-parallel over the 16 heads in groups of 4.
Each core computes its 4 heads' attention plus the corresponding slice of
the output projection; the host sums the 4 head-group partials per batch
(bf16 partials) and adds the output bias.

Per-core kernel (bf16 matmul inputs, fp32 PSUM):
  kT[256,2048], qT[256,2048] projections as before (gate sigmoid/sqrt(D)
  folded into the q scale, biases into the DVE PSUM->SBUF eviction).
  v[2048 pos, 4 heads, 65] with a ones column per head (denominator).

  Attention runs per (512-query block, head pair), heads alternating:
    QK^T: per kc chunk one matmul [K=64, M=128 kpos, F=512] into an
      S^T psum tile [128, 2, 512] (two kc chunks side by side); the two
      heads of a pair row-pack the PE array (rows 0-63 / 64-127).
    exp on ACT as [128,1024] activations (no max-subtraction).
    AV V-stationary: lhsT = v[kc][h] [128, 65], rhs = exp(S^T) [128,512]
      -> A^T psum [65 (64 dims + denom), 512 q] accumulated over 16 kc.
      F=512 streaming instead of the F=65 of the A-layout — 4x fewer
      PE rows and no PE transposes before the output projection.
    normalize: DVE reciprocal of the denom row -> DRAM bounce ->
      partition-broadcast DMA -> DVE multiply evicts A^T to SBUF bf16;
      odd heads land at partitions 64-127 via a small DMA shift.
  O-proj per 128-q chunk straight from the A^T tiles (no transposes);
  y stored bf16, summed on host in f32.
"""

import math
from collections import deque
from contextlib import ExitStack

import numpy as np

import concourse.bass as bass
import concourse.tile as tile
from concourse import mybir
from concourse.bass_utils import run_bass_kernel_spmd



B = 2
N = 2048
E = 1024
H = 16
D = 64
NCORES = 8
GROUPS = NCORES // B      # head-groups per batch
HG = H // GROUPS          # heads per core
DH = HG * D               # 256 head-dims per core
P = 128

F32 = mybir.dt.float32
BF16 = mybir.dt.bfloat16
AF = mybir.ActivationFunctionType
ALU = mybir.AluOpType

TRACE = False
LAST_RESULTS = None

KC = E // P            # 8 contraction chunks over the embed dim
MC = DH // P           # 2 partition chunks over this core's head dims
NB = N // 512          # 4 blocks of 512 positions
KB = N // P            # 16 key-position chunks
NPAIR = HG // 2        # head pairs per core
NQB = N // 512         # query 512-blocks


def _split_drain_waits(nc):
    """The installed walrus build accepts only ONE sync-wait per instruction
    (one NEURON_ISA_TPB_EVENTS slot), but Tile emits several on drains,
    matmuls, etc.  Hoist all but the last wait onto dedicated single-wait
    NOPs ahead of the instruction on the same engine (the lowering newer
    walrus performs itself)."""
    n = 0
    for fn in nc.m.functions:
        for bb in fn.blocks:
            insts = bb.instructions
            idx = 0
            while idx < len(insts):
                inst = insts[idx]
                si = inst.sync_info
                if si is not None and len(si.on_wait) > 1:
                    waits = list(si.on_wait)
                    nops = []
                    for w in waits[:-1]:
                        n += 1
                        nop = mybir.InstNoOp(
                            name=f"waitsplit-{n}",
                            engine=inst.engine,
                            sync_info=mybir.SyncInfo(on_wait=[w], on_update=[]),
                            bass_nofuse=True,
                        )
                        nc.register_instruction(nop)
                        nops.append(nop)
                    inst.sync_info = mybir.SyncInfo(
                        on_wait=[waits[-1]], on_update=list(si.on_update))
                    insts[idx:idx] = nops
                    idx += len(nops)
                idx += 1
    return n


def _build(reps=1):
    nc = bass.Bass()
    # inputs arrive partition-major ([128, chunk, free]) so each loads as a
    # single DMA with 32KB contiguous per-partition runs
    xqT = nc.dram_tensor("xqT", [P, KC, N], BF16, kind="ExternalInput")
    xkT = nc.dram_tensor("xkT", [P, KC, N], BF16, kind="ExternalInput")
    xvT = nc.dram_tensor("xvT", [P, KC, N], BF16, kind="ExternalInput")
    wqT = nc.dram_tensor("wqT", [P, KC, DH], BF16, kind="ExternalInput")
    wkT = nc.dram_tensor("wkT", [P, KC, DH], BF16, kind="ExternalInput")
    wvT = nc.dram_tensor("wvT", [P, KC, DH], BF16, kind="ExternalInput")
    woB = nc.dram_tensor("woB", [P, MC, E], BF16, kind="ExternalInput")
    qscale = nc.dram_tensor("qscale", [DH], F32, kind="ExternalInput")
    qbias = nc.dram_tensor("qbias", [DH], F32, kind="ExternalInput")
    kbias = nc.dram_tensor("kbias", [DH], F32, kind="ExternalInput")
    vbias = nc.dram_tensor("vbias", [DH], F32, kind="ExternalInput")
    y = nc.dram_tensor("y", [N, E], BF16, kind="ExternalOutput")
    # scratch rows for the denominator-reciprocal partition broadcast:
    # scr holds raw denominators, scr2 their reciprocals
    scr = nc.dram_tensor("scr", [NQB * HG, 512], F32, kind="Internal")
    scr2 = nc.dram_tensor("scr2", [NQB * HG, 512], F32, kind="Internal")

    with ExitStack() as ctx:
        tc = ctx.enter_context(tile.TileContext(nc))
        const = ctx.enter_context(tc.tile_pool(name="const", bufs=1))
        xpool = ctx.enter_context(tc.tile_pool(name="xpool", bufs=3))
        wpool = ctx.enter_context(tc.tile_pool(name="wpool", bufs=2))
        wqpool = ctx.enter_context(tc.tile_pool(name="wqpool", bufs=1))
        wopool = ctx.enter_context(tc.tile_pool(name="wopool", bufs=1))
        qkpool = ctx.enter_context(tc.tile_pool(name="qkpool", bufs=4))
        vpool = ctx.enter_context(tc.tile_pool(name="vpool", bufs=KB))
        ptpool = ctx.enter_context(tc.tile_pool(name="ptpool", bufs=12))
        attpool = ctx.enter_context(tc.tile_pool(name="attpool", bufs=4))
        tmppool = ctx.enter_context(tc.tile_pool(name="tmppool", bufs=2))
        stgpool = ctx.enter_context(tc.tile_pool(name="stgpool", bufs=4))
        rpool = ctx.enter_context(tc.tile_pool(name="rpool", bufs=2))
        dpool = ctx.enter_context(tc.tile_pool(name="dpool", bufs=4))
        bcpool = ctx.enter_context(tc.tile_pool(name="bcpool", bufs=2))
        ypool = ctx.enter_context(tc.tile_pool(name="ypool", bufs=2))
        # PSUM: stq 2x2 banks + pav 2x1 + ppp 2x1 -> exactly the 8 banks
        pstq = ctx.enter_context(tc.tile_pool(name="pstq", bufs=2,
                                              space="PSUM"))
        pav = ctx.enter_context(tc.tile_pool(name="pav", bufs=2,
                                             space="PSUM"))
        ppp = ctx.enter_context(tc.tile_pool(name="ppp", bufs=2,
                                             space="PSUM"))

        # ---- constant DMAs off the critical queue ----
        qs_sb = const.tile([P, MC], F32, name="qs")
        nc.scalar.dma_start(out=qs_sb,
                            in_=qscale[:].rearrange("(c p) -> p c", p=P))
        qb_sb = const.tile([P, MC], F32, name="qb")
        nc.scalar.dma_start(out=qb_sb,
                            in_=qbias[:].rearrange("(c p) -> p c", p=P))
        kb_sb = const.tile([P, MC], F32, name="kb")
        nc.scalar.dma_start(out=kb_sb,
                            in_=kbias[:].rearrange("(c p) -> p c", p=P))
        vb_ap = vbias[:]
        vb_bc = const.tile([P, DH], F32, name="vb")
        nc.gpsimd.dma_start(out=vb_bc, in_=bass.AP(
            tensor=vb_ap.tensor, offset=vb_ap.offset, ap=[[0, P]] + vb_ap.ap))


        def load_x(x_dram, engine):
            # 4 pieces into one tile: consumers start before the tail lands
            t = xpool.tile([P, KC, N], BF16, name="xs")
            for i in range(4):
                engine.dma_start(out=t[:, 2 * i:2 * i + 2, :],
                                 in_=x_dram[:, 2 * i:2 * i + 2, :])
            return [t[:, kc, :] for kc in range(KC)]

        def load_w(w_dram, pool, tag, engine):
            t = pool.tile([P, KC, DH], BF16, name=tag)
            engine.dma_start(out=t, in_=w_dram[:, :, :])
            return [t[:, kc, :] for kc in range(KC)]

        def emit_body():
            # xk then xq serialized on the SP queue (K proj gates everything);
            # xv concurrent on the gpsimd queue; weights on the ACT queue
            xk = load_x(xkT, nc.sync)
            xq = load_x(xqT, nc.sync)
            xv = load_x(xvT, nc.gpsimd)
            wk_c = load_w(wkT, wpool, "ws", nc.scalar)
            wq_c = load_w(wqT, wqpool, "wq", nc.scalar)
            wv_c = load_w(wvT, wpool, "ws", nc.scalar)
            wo_t = wopool.tile([P, MC, E], BF16, name="wo")
            nc.scalar.dma_start(out=wo_t, in_=woB[:, :, :])
            wo_sb = [wo_t[:, c, :] for c in range(MC)]

            # ---- projections (evictions on DVE) ----
            kT = [qkpool.tile([P, N], BF16, name="kt") for _ in range(MC)]
            qT = [qkpool.tile([P, N], BF16, name="qt") for _ in range(MC)]

            def proj_evict(pt, out_sb, c, nb, scale_sb, bias_sb):
                if scale_sb is None:
                    nc.vector.tensor_scalar_add(
                        out=out_sb[c][:, nb * 512:(nb + 1) * 512], in0=pt,
                        scalar1=bias_sb[:, c:c + 1])
                else:
                    nc.vector.tensor_scalar(
                        out=out_sb[c][:, nb * 512:(nb + 1) * 512], in0=pt,
                        scalar1=scale_sb[:, c:c + 1], scalar2=bias_sb[:, c:c + 1],
                        op0=ALU.mult, op1=ALU.add)

            def proj_group(xs, w_c, out_sb, c, nb, scale_sb, bias_sb):
                pt = ppp.tile([P, 512], F32, name="pp")
                for kc in range(KC):
                    nc.tensor.matmul(
                        pt,
                        lhsT=w_c[kc][:, c * P:(c + 1) * P],
                        rhs=xs[kc][:, nb * 512:(nb + 1) * 512],
                        start=(kc == 0), stop=(kc == KC - 1))
                proj_evict(pt, out_sb, c, nb, scale_sb, bias_sb)

            def proj_wave(xs, w_c, out_sb, groups, scale_sb, bias_sb, pools):
                # kc-outer so the matmuls chase the arriving x DMA pieces.
                # pstq-pool tiles keep their native [P, 2, 512] shape (mixed
                # sizes would fragment the pool); the wave uses half of each.
                pts = [pool.tile([P, 512], F32, name="pp") if pool is ppp
                       else pool.tile([P, 2, 512], F32, name="stq")[:, 0, :]
                       for pool in pools]
                for kc in range(KC):
                    for g, (c, nb) in enumerate(groups):
                        nc.tensor.matmul(
                            pts[g],
                            lhsT=w_c[kc][:, c * P:(c + 1) * P],
                            rhs=xs[kc][:, nb * 512:(nb + 1) * 512],
                            start=(kc == 0), stop=(kc == KC - 1))
                for g, (c, nb) in enumerate(groups):
                    proj_evict(pts[g], out_sb, c, nb, scale_sb, bias_sb)

            v_sb = []

            def vproj_group(m):
                vt = vpool.tile([P, HG, D + 1], BF16, name="vt")
                nc.gpsimd.memset(vt[:, :, D:D + 1], 1.0)
                pv = ppp.tile([P, 512], F32, name="pp")[:, :DH]
                for kc in range(KC):
                    nc.tensor.matmul(
                        pv,
                        lhsT=xv[kc][:, m * P:(m + 1) * P],
                        rhs=wv_c[kc],
                        start=(kc == 0), stop=(kc == KC - 1))
                nc.vector.tensor_add(
                    out=vt[:, :, 0:D],
                    in0=pv.rearrange("p (h d) -> p h d", h=HG),
                    in1=vb_bc.rearrange("p (h d) -> p h d", h=HG))
                v_sb.append(vt)

            # K proj in two kc-outer waves of 4 (chases the xk DMA pieces;
            # borrows idle attention PSUM banks during the prologue), then
            # the first Q proj block likewise; remaining Q blocks are fills
            proj_wave(xk, wk_c, kT, [(0, 0), (0, 1), (1, 0), (1, 1)],
                      None, kb_sb, [ppp, ppp, pstq, pstq])
            proj_wave(xk, wk_c, kT, [(0, 2), (0, 3), (1, 2), (1, 3)],
                      None, kb_sb, [ppp, ppp, pstq, pstq])
            proj_wave(xq, wq_c, qT, [(0, 0), (1, 0)],
                      qs_sb, qb_sb, [ppp, ppp])

            kq_fills = deque()
            for nb in range(1, NB):
                for c in range(MC):
                    kq_fills.append(lambda c=c, nb=nb: proj_group(
                        xq, wq_c, qT, c, nb, qs_sb, qb_sb))
            o_fills = deque()

            # ---- attention: units of (512-query block, head pair) ----
            units = [(qb, pr) for qb in range(NQB) for pr in range(NPAIR)]
            att_tiles = {}          # qb -> [att_c2 tile for c2 in MC]

            def get_att(qb):
                if qb not in att_tiles:
                    att_tiles[qb] = [attpool.tile([P, 512], BF16, name="att")
                                     for _ in range(MC)]
                return att_tiles[qb]

            def emit_unit(u, first, last=False):
                qb, pr = u
                q0 = qb * 512
                avp = {}
                pend = deque()      # AV groups pipelined one kc-pair behind
                for h2 in range(2):     # head pr*2 + h2 at partitions 64*h2
                    avp[h2] = pav.tile([P, 512], F32, name="av")

                def emit_av(j, h2, ptile):
                    h = 2 * pr + h2
                    for t in range(2):
                        kc = 2 * j + t
                        nc.tensor.matmul(
                            avp[h2][0:D + 1, :],
                            lhsT=v_sb[kc][:, h, :],
                            rhs=ptile[:, t, :],
                            start=(kc == 0), stop=(kc == KB - 1),
                            skip_group_check=True)

                for j in range(KB // 2):    # kc pair (2j, 2j+1)
                    if first:
                        # V proj must stay ahead of the AV consumption
                        vproj_group(2 * j)
                        vproj_group(2 * j + 1)
                    for h2 in range(2):
                        off = h2 * D
                        stq = pstq.tile([P, 2, 512], F32, name="stq")
                        for t in range(2):
                            kc = 2 * j + t
                            nc.tensor.matmul(
                                stq[:, t, :],
                                lhsT=kT[pr][off:off + D, kc * P:(kc + 1) * P],
                                rhs=qT[pr][off:off + D, q0:q0 + 512],
                                start=True, stop=True)
                        ptile = ptpool.tile([P, 2, 512], BF16, name="pt")
                        nc.scalar.activation(out=ptile, in_=stq, func=AF.Exp)
                        pend.append((j, h2, ptile))
                    # AVs of the previous kc pair: their exps completed
                    # while this pair's QK matmuls streamed
                    while len(pend) > 2:
                        emit_av(*pend.popleft())
                    if not first:
                        pop_fills(1)
                while pend:
                    emit_av(*pend.popleft())
                if last:
                    # while the eviction chain's latency plays out, the PE
                    # runs the c2=0 halves of this qb's output projection
                    # (att[0] was evicted during this unit's attention),
                    # accumulating into the now-idle attention PSUM banks
                    oparts = emit_oproj_part1(qb)
                # normalize + evict A^T (denominator row D).  Stage both the
                # dims and the denom row out of PSUM immediately (releases
                # the accumulation bank); the reciprocal runs 128-lane via a
                # DRAM reshape bounce, then a partition-broadcast DMA feeds
                # the normalizing multiply.
                att = get_att(qb)[pr]
                sgs, rts = {}, {}
                for h2 in range(2):
                    sg = stgpool.tile([D, 512], BF16, name="sg")
                    nc.vector.tensor_copy(out=sg, in_=avp[h2][0:D, :])
                    rt = rpool.tile([1, 512], F32, name="rt")
                    nc.vector.tensor_copy(out=rt, in_=avp[h2][D:D + 1, :])
                    sgs[h2], rts[h2] = sg, rt
                for h2 in range(2):
                    row = qb * HG + pr * 2 + h2
                    nc.gpsimd.dma_start(out=scr[row, :], in_=rts[h2])
                    dn = dpool.tile([P, 4], F32, name="dn")
                    nc.gpsimd.dma_start(
                        out=dn, in_=scr[row, :].rearrange("(p j) -> p j", p=P))
                    rc = dpool.tile([P, 4], F32, name="rc")
                    nc.vector.reciprocal(out=rc, in_=dn)
                    nc.gpsimd.dma_start(
                        out=scr2[row, :].rearrange("(p j) -> p j", p=P), in_=rc)
                    bc = bcpool.tile([D, 512], F32, name="bc")
                    sap = scr2[row, :]
                    nc.gpsimd.dma_start(out=bc, in_=bass.AP(
                        tensor=sap.tensor, offset=sap.offset,
                        ap=[[0, D]] + sap.ap))
                    if h2 == 0:
                        nc.vector.tensor_tensor(
                            out=att[0:D, :], in0=sgs[h2], in1=bc, op=ALU.mult)
                    else:
                        tmp = tmppool.tile([D, 512], BF16, name="tmp")
                        nc.vector.tensor_tensor(
                            out=tmp, in0=sgs[h2], in1=bc, op=ALU.mult)
                        nc.sync.dma_start(out=att[D:P, :], in_=tmp)
                if last:
                    emit_oproj_part2(qb, oparts)

            def emit_oproj_part1(qb):
                # c2=0 partial accumulations for all 8 (qc, nn) blocks of
                # this qb, spread over the ppp/pav/pstq pools (the attention
                # banks are idle once the final AVs and exps retire)
                att0 = get_att(qb)[0]
                oparts = []
                holders = [ppp.tile([P, 512], F32, name="pp"),
                           ppp.tile([P, 512], F32, name="pp"),
                           pav.tile([P, 512], F32, name="av"),
                           pav.tile([P, 512], F32, name="av")]
                for t2 in range(2):
                    st = pstq.tile([P, 2, 512], F32, name="stq")
                    holders.append(st[:, 0, :])
                    holders.append(st[:, 1, :])
                for i, (qc, nn) in enumerate(
                        (qc, nn) for qc in range(4) for nn in range(2)):
                    pt = holders[i]
                    nc.tensor.matmul(
                        pt, lhsT=att0[:, qc * P:(qc + 1) * P],
                        rhs=wo_sb[0][:, nn * 512:(nn + 1) * 512],
                        start=True, stop=False, skip_group_check=True)
                    oparts.append((qc, nn, pt))
                return oparts

            def emit_oproj_part2(qb, oparts):
                att1 = get_att(qb)[1]
                for qc, nn, pt in oparts:
                    nc.tensor.matmul(
                        pt, lhsT=att1[:, qc * P:(qc + 1) * P],
                        rhs=wo_sb[1][:, nn * 512:(nn + 1) * 512],
                        start=False, stop=True, skip_group_check=True)
                # evict per (qc, nn): ACT is idle after the final exps
                ydone = {}
                for qc, nn, pt in oparts:
                    yt = ydone.setdefault(
                        qc, ypool.tile([P, E], BF16, name="yt"))
                    nc.scalar.copy(out=yt[:, nn * 512:(nn + 1) * 512], in_=pt)
                for qc, yt in ydone.items():
                    q0 = qb * 512 + qc * P
                    nc.sync.dma_start(out=y[q0:q0 + P, :], in_=yt)

            def emit_oproj(qb, qc, tail=False):
                att = get_att(qb)
                yt = ypool.tile([P, E], BF16, name="yt")
                for nn in range(2):
                    py = ppp.tile([P, 512], F32, name="pp")
                    for c2 in range(MC):
                        nc.tensor.matmul(
                            py, lhsT=att[c2][:, qc * P:(qc + 1) * P],
                            rhs=wo_sb[c2][:, nn * 512:(nn + 1) * 512],
                            start=(c2 == 0), stop=(c2 == MC - 1))
                    if tail:    # ACT is idle once the last exp has issued
                        nc.scalar.copy(out=yt[:, nn * 512:(nn + 1) * 512], in_=py)
                    else:
                        nc.vector.tensor_copy(
                            out=yt[:, nn * 512:(nn + 1) * 512], in_=py)
                q0 = qb * 512 + qc * P
                nc.sync.dma_start(out=y[q0:q0 + P, :], in_=yt)

            def pop_fills(budget):
                while budget > 0:
                    if kq_fills:
                        kq_fills.popleft()()
                    elif o_fills:
                        o_fills.popleft()()
                    else:
                        break
                    budget -= 1

            for i, u in enumerate(units):
                emit_unit(u, first=(i == 0), last=(i == len(units) - 1))
                qb, pr = u
                if pr == NPAIR - 1 and qb != NQB - 1:
                    for qc in range(4):
                        o_fills.append(
                            lambda qb=qb, qc=qc: emit_oproj(qb, qc))
            while kq_fills or o_fills:
                pop_fills(1)

        for _ in range(reps):
            emit_body()

    _split_drain_waits(nc)
    return nc


_CACHE = {}


def _get_nc():
    if "nc" not in _CACHE:
        _CACHE["nc"] = _build()
    return _CACHE["nc"]


def make_in_maps(query, key, value, Wq, bq, Wk, bk, Wv, bv, Wo, bo, gate):
    query = np.asarray(query, np.float32)
    key = np.asarray(key, np.float32)
    value = np.asarray(value, np.float32)
    Wq = np.asarray(Wq, np.float32)
    Wk = np.asarray(Wk, np.float32)
    Wv = np.asarray(Wv, np.float32)
    Wo = np.asarray(Wo, np.float32)
    bq = np.asarray(bq, np.float32)
    bk = np.asarray(bk, np.float32)
    bv = np.asarray(bv, np.float32)
    gate = np.asarray(gate, np.float32)

    scale_h = (1.0 / (1.0 + np.exp(-gate.astype(np.float64)))
               / math.sqrt(D)).astype(np.float32)

    def pmajor(a, np_, p=P):
        # [np_*p, free] -> [p, np_, free] (partition-major for 1-DMA loads)
        return np.ascontiguousarray(
            a.reshape(np_, p, a.shape[1]).transpose(1, 0, 2))

    xq_b = [pmajor(query[b].T, KC) for b in range(B)]
    xk_b = [pmajor(key[b].T, KC) for b in range(B)]
    xv_b = [pmajor(value[b].T, KC) for b in range(B)]

    in_maps = []
    for core in range(NCORES):
        b, g = divmod(core, GROUPS)
        rows = slice(g * DH, (g + 1) * DH)
        qs = np.repeat(scale_h[g * HG:(g + 1) * HG], D)
        in_maps.append({
            "xqT": xq_b[b], "xkT": xk_b[b], "xvT": xv_b[b],
            "wqT": pmajor(Wq[rows].T, KC),
            "wkT": pmajor(Wk[rows].T, KC),
            "wvT": pmajor(Wv[rows].T, KC),
            "woB": pmajor(Wo[:, rows].T, MC),
            "qscale": np.ascontiguousarray(qs),
            "qbias": np.ascontiguousarray(bq[rows] * qs),
            "kbias": np.ascontiguousarray(bk[rows]),
            "vbias": np.ascontiguousarray(bv[rows]),
        })

    from concourse import mybir as _mb
    bf = _mb.dt.np(_mb.dt.bfloat16)
    for m in in_maps:
        for k in ("xqT", "xkT", "xvT", "wqT", "wkT", "wvT", "woB"):
            m[k] = m[k].astype(bf)
    return in_maps


def assemble(results, query, key, value, Wq, bq, Wk, bk, Wv, bv, Wo, bo, gate):
    bo = np.asarray(bo, np.float32)
    out = np.empty((B, N, E), np.float32)
    for b in range(B):
        acc = results[b * GROUPS]["y"].astype(np.float32)
        for g in range(1, GROUPS):
            acc = acc + results[b * GROUPS + g]["y"].astype(np.float32)
        out[b] = acc + bo
    return out


def kernel(**inputs):
    global LAST_RESULTS
    in_maps = make_in_maps(**inputs)
    res = run_bass_kernel_spmd(_get_nc(), in_maps, list(range(NCORES)),
                               trace=TRACE)
    LAST_RESULTS = res
    return assemble(res.results, **inputs)
